# revision 1
# baseline (speedup 1.0000x reference)
"""Fused cross-attention kernel for TRN2, sharded over 8 NeuronCores.

Sharding: core = 2*b + g  (b = batch 0..3 data-parallel, g = head-group 0..1
tensor-parallel over heads: heads g*8..g*8+7, i.e. columns g*512..(g+1)*512 of
Wq/Wk/Wv and rows g*512..(g+1)*512 of Wo). Each core computes a partial
out = softmax((x@Wq)(ctx@Wk)^T/sqrt(d)) (ctx@Wv) @ Wo_slice for its batch;
the host sums the two head-group partials per batch and adds bo.

On-device layout (per core), all matmul operands bf16, PSUM accum fp32:
  Q^T = (Wq_g)^T x^T    [512, 2048]  (4 sbuf tiles [128, 2048], head-pair per
  K^T = (Wk_g)^T ctx^T  [512, 2048]   tile: head A rows 0-63, head B 64-127)
  V   = ctx @ Wv_g      [2048, 512]  (16 m-tiles [128, 8*65]: per head 64 V
                                      cols + a ones column for softmax sums)
  Attention per head-pair, per n-block(512): S^T tiles [128m, 512n] via
  row-packed K=64 matmuls (2 heads concurrent in the PE array); exp on
  ScalarE (scale=1/8, bias=log-mask[m], bf16 out);
  O^T[65, n] += [V|1]^T @ expS^T accumulated over 16 m-tiles in PSUM
  (row 64 = softmax sums). Normalize: reciprocal_approx_fast of row 64,
  DMA partition-broadcast (0-stride src), DVE multiplies; head B's rows are
  DMA-shifted to partitions 64-127 of the pair O^T tile.
  out = (O^T_norm).T @ Wo_g  via lhsT = O^T_norm. The inner loop is
  software-pipelined: S(t+1) is emitted before attnV(t) so the PE never
  waits on the ScalarE exp of tile t.
"""
import numpy as np

B, N, M = 4, 2048, 2048
DQ = 1024
DC = 1024
H = 16
DH = 64
INNER = 1024
HG = 2            # head groups (tensor parallel)
HPC = H // HG     # heads per core
CI = HPC * DH     # 512 inner dims per core
NCORES = 8
PT = 128          # partition tile
NB = 512          # n-block
KT_DQ = DQ // PT  # 8 contraction tiles for projections
MT = M // PT      # 16 m-tiles
NT = N // PT      # 16 n-tiles
SCALE = DH ** -0.5

_CACHE = {}


def _build_nc():
    import concourse.bass as bass
    import concourse.mybir as mybir
    import concourse.tile as tile
    from concourse import bacc

    F32 = mybir.dt.float32
    BF16 = mybir.dt.bfloat16
    EXP = mybir.ActivationFunctionType.Exp

    nc = bacc.Bacc("TRN2", target_bir_lowering=False, debug=False,
                   num_devices=NCORES)

    xT_d = nc.dram_tensor("xT", [DQ, N], BF16, kind="ExternalInput")
    ctxT_d = nc.dram_tensor("ctxT", [DC, M], BF16, kind="ExternalInput")
    wq_d = nc.dram_tensor("wq", [DQ, CI], BF16, kind="ExternalInput")
    wk_d = nc.dram_tensor("wk", [DC, CI], BF16, kind="ExternalInput")
    wv_d = nc.dram_tensor("wv", [DC, CI], BF16, kind="ExternalInput")
    wo_d = nc.dram_tensor("wo", [CI, INNER], BF16, kind="ExternalInput")
    mb_d = nc.dram_tensor("maskb", [MT, PT], F32, kind="ExternalInput")
    out_d = nc.dram_tensor("out", [N, INNER], F32, kind="ExternalOutput")

    with tile.TileContext(nc) as tc:
      with tc.tile_pool(name="persist", bufs=1) as pp:
        kt = [pp.tile([PT, M], BF16, tag=f"kt{p}", name=f"kt{p}")
              for p in range(4)]
        vt = [pp.tile([PT, HPC * (DH + 1)], BF16, tag=f"vt{t}", name=f"vt{t}")
              for t in range(MT)]
        mask_t = pp.tile([PT, MT], F32, tag="mask")
        for t in range(MT):
            nc.sync.dma_start(mask_t[:, t:t + 1], mb_d[t, :])

        # ---- Phase A: K^T and V from ctx^T (wq/qt persist into C) ----
        with tc.tile_pool(name="qt_scope", bufs=1) as pq:
            qt = [pq.tile([PT, N], BF16, tag=f"qt{p}", name=f"qt{p}")
                  for p in range(4)]
            wq_t = [pq.tile([PT, CI], BF16, tag=f"wq{k}", name=f"wq{k}")
                    for k in range(KT_DQ)]
            with (
                tc.tile_pool(name="phA", bufs=1) as pa,
                tc.tile_pool(name="phA_s", bufs=2) as pas,
                tc.tile_pool(name="psA", bufs=4, space="PSUM") as psA,
            ):
                wk_t = [pa.tile([PT, CI], BF16, tag=f"wk{k}", name=f"wk{k}")
                        for k in range(KT_DQ)]
                wv_t = [pa.tile([PT, CI], BF16, tag=f"wv{k}", name=f"wv{k}")
                        for k in range(KT_DQ)]
                # startup-critical DMAs first: wk + ctx quarter 0
                for k in range(KT_DQ):
                    nc.sync.dma_start(wk_t[k][:], wk_d[k * PT:(k + 1) * PT, :])
                ctx0 = []
                for k in range(KT_DQ):
                    c = pas.tile([PT, NB], BF16, tag=f"ctx{k}", name=f"ctx{k}")
                    nc.sync.dma_start(c[:], ctxT_d[k * PT:(k + 1) * PT, 0:NB])
                    ctx0.append(c)
                for k in range(KT_DQ):
                    nc.sync.dma_start(wv_t[k][:], wv_d[k * PT:(k + 1) * PT, :])
                    nc.sync.dma_start(wq_t[k][:], wq_d[k * PT:(k + 1) * PT, :])
                for q in range(M // NB):
                    mq = slice(q * NB, (q + 1) * NB)
                    if q == 0:
                        ctx = ctx0
                    else:
                        ctx = []
                        for k in range(KT_DQ):
                            c = pas.tile([PT, NB], BF16, tag=f"ctx{k}",
                                         name=f"ctx{k}")
                            nc.sync.dma_start(
                                c[:], ctxT_d[k * PT:(k + 1) * PT, mq])
                            ctx.append(c)
                    for p in range(4):
                        ps = psA.tile([PT, NB], F32, tag="psA")
                        for k in range(KT_DQ):
                            nc.tensor.matmul(
                                ps[:], wk_t[k][:, p * PT:(p + 1) * PT],
                                ctx[k][:],
                                start=(k == 0), stop=(k == KT_DQ - 1))
                        nc.vector.tensor_copy(kt[p][:, mq], ps[:])
                    for ti in range(NB // PT):
                        t = q * (NB // PT) + ti
                        ps = psA.tile([PT, CI], F32, tag="psA")
                        for k in range(KT_DQ):
                            nc.tensor.matmul(
                                ps[:], ctx[k][:, ti * PT:(ti + 1) * PT],
                                wv_t[k][:],
                                start=(k == 0), stop=(k == KT_DQ - 1))
                        dst = vt[t][:].rearrange("p (h c) -> p h c", c=DH + 1)
                        nc.vector.tensor_copy(
                            dst[:, :, 0:DH],
                            ps[:].rearrange("p (h c) -> p h c", c=DH))
                        nc.vector.memset(dst[:, :, DH:DH + 1], 1.0)

            # -------- Phase C: attention, with Q^T chains interleaved -----
            with tc.tile_pool(name="ot_pool", bufs=1) as po:
                ot = [po.tile([PT, N], BF16, tag=f"ot{p}", name=f"ot{p}")
                      for p in range(4)]

                with (
                    tc.tile_pool(name="attn_sb", bufs=3) as asb,
                    tc.tile_pool(name="attn_small", bufs=2) as asmall,
                    tc.tile_pool(name="xt_s", bufs=2) as pbs,
                    tc.tile_pool(name="ps_s", bufs=2, space="PSUM") as ps_s,
                    tc.tile_pool(name="ps_o", bufs=2, space="PSUM") as ps_o,
                ):
                    def emit_qchain(p, q, xt):
                        # Q^T chain through a shared ps_s slot (uses half)
                        nq = slice(q * NB, (q + 1) * NB)
                        ps = ps_s.tile([PT, 2 * NB], F32, tag="sps",
                                       name="sps")
                        for k in range(KT_DQ):
                            nc.tensor.matmul(
                                ps[:, 0:NB], wq_t[k][:, p * PT:(p + 1) * PT],
                                xt[k][:],
                                start=(k == 0), stop=(k == KT_DQ - 1))
                        nc.vector.tensor_copy(qt[p][:, nq], ps[:, 0:NB])

                    def emit_xt_dma(q):
                        nq = slice(q * NB, (q + 1) * NB)
                        xt = []
                        for k in range(KT_DQ):
                            c = pbs.tile([PT, NB], BF16, tag=f"xt{k}",
                                         name=f"xt{k}")
                            nc.sync.dma_start(
                                c[:], xT_d[k * PT:(k + 1) * PT, nq])
                            xt.append(c)
                        return xt

                    def emit_normalize(prev):
                        p, jq, oA, oB = prev
                        # plain copies handle the partition shift 64->0;
                        # reciprocal_approx_fast must stay partition-aligned
                        sums = asmall.tile([1, 2 * NB], F32, tag="sums",
                                           name="sums")
                        nc.vector.tensor_copy(sums[0:1, 0:NB],
                                              oA[DH:DH + 1, :])
                        nc.vector.tensor_copy(sums[0:1, NB:2 * NB],
                                              oB[DH:DH + 1, :])
                        rr = asmall.tile([1, 2 * NB], F32, tag="rr",
                                         name="rr")
                        nc.vector.reciprocal_approx_fast(
                            rr[0:1, :], sums[0:1, :])
                        bcs = asmall.tile([DH, 2 * NB], F32, tag="bcs",
                                          name="bcs")
                        nc.gpsimd.partition_broadcast(
                            bcs[:], rr[0:1, :])
                        nc.vector.tensor_mul(
                            ot[p][0:DH, jq], oA[0:DH, :], bcs[:, 0:NB])
                        tmpB = asmall.tile([DH, NB], BF16, tag="tmpB",
                                           name="tmpB")
                        nc.vector.tensor_mul(
                            tmpB[:], oB[0:DH, :], bcs[:, NB:2 * NB])
                        nc.sync.dma_start(ot[p][DH:2 * DH, jq], tmpB[:])

                    def emit_s(p, jq, t):
                        sps = ps_s.tile([PT, 2 * NB], F32, tag="sps",
                                        name="sps")
                        nc.tensor.matmul(
                            sps[:, 0:NB],
                            kt[p][0:DH, t * PT:(t + 1) * PT],
                            qt[p][0:DH, jq], start=True, stop=True)
                        nc.tensor.matmul(
                            sps[:, NB:2 * NB],
                            kt[p][DH:2 * DH, t * PT:(t + 1) * PT],
                            qt[p][DH:2 * DH, jq], start=True, stop=True)
                        pe = asb.tile([PT, 2 * NB], BF16, tag="pe", name="pe")
                        nc.scalar.activation(pe[:], sps[:], EXP,
                                             bias=mask_t[:, t:t + 1],
                                             scale=SCALE)
                        return pe

                    def emit_av(pes, oA, oB, hA, hB, t):
                        nc.tensor.matmul(
                            oA[:],
                            vt[t][:, hA * (DH + 1):(hA + 1) * (DH + 1)],
                            pes[:, 0:NB],
                            start=(t == 0), stop=(t == MT - 1))
                        nc.tensor.matmul(
                            oB[:],
                            vt[t][:, hB * (DH + 1):(hB + 1) * (DH + 1)],
                            pes[:, NB:2 * NB],
                            start=(t == 0), stop=(t == MT - 1))

                    # Q^T quarter 0 for all pairs first (allows attention
                    # p0/j0 to start); remaining quarters spread inside the
                    # first pair's attention blocks.
                    xt = emit_xt_dma(0)
                    for p in range(4):
                        emit_qchain(p, 0, xt)

                    prev = None
                    pending_q = 1
                    pending_xt = None
                    for p in range(4):
                        hA, hB = 2 * p, 2 * p + 1
                        for j in range(N // NB):
                            jq = slice(j * NB, (j + 1) * NB)
                            oA = ps_o.tile([DH + 1, NB], F32, tag="oA",
                                           name="oA")
                            oB = ps_o.tile([DH + 1, NB], F32, tag="oB",
                                           name="oB")
                            pes = [None] * MT
                            for t in range(MT):
                                pes[t] = emit_s(p, jq, t)
                                if t == 2 and prev is not None:
                                    emit_normalize(prev)
                                    prev = None
                                if t >= 1:
                                    emit_av(pes[t - 1], oA, oB, hA, hB, t - 1)
                                    pes[t - 1] = None
                                # spread remaining Q^T quarters into the
                                # first pair's blocks (one chain per 4 iters)
                                if p == 0 and pending_q < 4:
                                    if t == 3:
                                        pending_xt = emit_xt_dma(pending_q)
                                    elif t in (5, 8, 11, 14):
                                        pp_i = (t - 5) // 3
                                        emit_qchain(pp_i, pending_q,
                                                    pending_xt)
                                        if t == 14:
                                            pending_q += 1
                                            pending_xt = None
                            emit_av(pes[MT - 1], oA, oB, hA, hB, MT - 1)
                            prev = (p, jq, oA, oB)
                    emit_normalize(prev)

                # ---------------- Phase D: out = O^T.T @ Wo ----------------
                with (
                    tc.tile_pool(name="phD", bufs=1) as pd,
                    tc.tile_pool(name="phD_out", bufs=3) as pdo,
                    tc.tile_pool(name="psD", bufs=4, space="PSUM") as psD,
                ):
                    wo_t = [pd.tile([PT, INNER], BF16, tag=f"wo{k}",
                                    name=f"wo{k}") for k in range(4)]
                    for k in range(4):
                        nc.sync.dma_start(wo_t[k][:],
                                          wo_d[k * PT:(k + 1) * PT, :])
                    for nt in range(NT):
                        for c in range(INNER // NB):
                            ps = psD.tile([PT, NB], F32, tag="psD")
                            for k in range(4):
                                nc.tensor.matmul(
                                    ps[:],
                                    ot[k][:, nt * PT:(nt + 1) * PT],
                                    wo_t[k][:, c * NB:(c + 1) * NB],
                                    start=(k == 0), stop=(k == 3))
                            ob = pdo.tile([PT, NB], F32, tag="ob")
                            nc.vector.tensor_copy(ob[:], ps[:])
                            nc.sync.dma_start(
                                out_d[nt * PT:(nt + 1) * PT,
                                      c * NB:(c + 1) * NB],
                                ob[:])

    nc.compile()
    return nc


def _get_nc():
    if "nc" not in _CACHE:
        _CACHE["nc"] = _build_nc()
    return _CACHE["nc"]


def make_in_maps(x, context, mask, Wq, Wk, Wv, Wo):
    import ml_dtypes
    bf16 = ml_dtypes.bfloat16
    x = np.asarray(x, np.float32)
    context = np.asarray(context, np.float32)
    mask = np.asarray(mask)
    maskb = np.where(mask, np.float32(0.0),
                     np.float32(-1e30)).astype(np.float32)
    wqs, wks, wvs, wos = [], [], [], []
    for g in range(HG):
        cs = slice(g * CI, (g + 1) * CI)
        wqs.append(np.ascontiguousarray(
            np.asarray(Wq, np.float32)[:, cs].astype(bf16)))
        wks.append(np.ascontiguousarray(
            np.asarray(Wk, np.float32)[:, cs].astype(bf16)))
        wvs.append(np.ascontiguousarray(
            np.asarray(Wv, np.float32)[:, cs].astype(bf16)))
        wos.append(np.ascontiguousarray(
            np.asarray(Wo, np.float32)[cs, :].astype(bf16)))
    in_maps = []
    for b in range(B):
        xT = np.ascontiguousarray(x[b].T.astype(bf16))
        ctxT = np.ascontiguousarray(context[b].T.astype(bf16))
        mb = np.ascontiguousarray(maskb[b].reshape(MT, PT))
        for g in range(HG):
            in_maps.append({
                "xT": xT, "ctxT": ctxT,
                "wq": wqs[g], "wk": wks[g], "wv": wvs[g], "wo": wos[g],
                "maskb": mb,
            })
    return in_maps


def combine(results, bo):
    bo = np.asarray(bo, np.float32)
    out = np.empty((B, N, INNER), np.float32)
    for b in range(B):
        out[b] = (results[2 * b]["out"] + results[2 * b + 1]["out"]
                  + bo[None, :])
    return out


def kernel(x, context, mask, Wq, Wk, Wv, Wo, bo):
    from concourse import bass2jax
    nc = _get_nc()
    in_maps = make_in_maps(x, context, mask, Wq, Wk, Wv, Wo)
    results = bass2jax.run_bass_via_pjrt(nc, in_maps, n_cores=NCORES)
    return combine(results, bo)



# revision 5
# speedup vs baseline: 1.1473x; 1.1473x over previous
"""Fused cross-attention kernel for TRN2, sharded over 8 NeuronCores.

Sharding: core = 2*b + g  (b = batch 0..3 data-parallel, g = head-group 0..1
tensor-parallel over heads: heads g*8..g*8+7, i.e. columns g*512..(g+1)*512 of
Wq/Wk/Wv and rows g*512..(g+1)*512 of Wo). Each core computes a partial
out = softmax((x@Wq)(ctx@Wk)^T/sqrt(d)) (ctx@Wv) @ Wo_slice for its batch;
the host sums the two head-group partials per batch and adds bo.

Schedule: the ScalarE exp stream (256 x [128,1024] activations ~ 285us) is
the critical resource; everything else hides under it.  Loops run j (n-block)
outer, pair inner, m-tile innermost.  All non-attention PE work (K^T/V/Q
projections, out = O^T.T@Wo) is emitted through a work queue that drips ~1
matmul per t-step into the PE queue, plus watermark draining so the first
j-block can start ~15us in while K/V production continues underneath.
S pairs are row-tiled (heads at PE row tiles 0/64) and stream concurrently.
Mask is folded into V and the ones-column (masked rows contribute 0 to both
numerator and softmax sum), so exp needs no bias operand.  Normalize uses
DVE + a 0-stride DMA partition-broadcast (no gpsimd).
"""
import numpy as np

B, N, M = 4, 2048, 2048
DQ = 1024
DC = 1024
H = 16
DH = 64
INNER = 1024
HG = 2            # head groups (tensor parallel)
HPC = H // HG     # heads per core
CI = HPC * DH     # 512 inner dims per core
NCORES = 8
PT = 128          # partition tile
NB = 512          # n-block
KT_DQ = DQ // PT  # 8 contraction tiles for projections
MT = M // PT      # 16 m-tiles
NT = N // PT      # 16 n-tiles
NJ = N // NB      # 4 n-blocks
SCALE = DH ** -0.5

_CACHE = {}


class WorkQueue:
    """Ordered generators of background PE work, dripped into the emission
    stream.  pump(n) advances n yield-units; drain(tag) runs until the
    generator registered under tag has completed."""

    def __init__(self):
        self.items = []      # list of (tag, generator)
        self.done = set()
        self.active = None   # (tag, gen)

    def add(self, tag, gen):
        self.items.append((tag, gen))

    def _step(self):
        # advance the current generator by one unit; True if work remains
        if self.active is None:
            if not self.items:
                return False
            self.active = self.items.pop(0)
        tag, gen = self.active
        try:
            next(gen)
        except StopIteration:
            self.done.add(tag)
            self.active = None
        return True

    def pump(self, n):
        for _ in range(n):
            if not self._step():
                return

    def drain(self, tag):
        while tag not in self.done:
            if not self._step():
                raise RuntimeError(f"work item {tag} never registered")

    def drain_all(self):
        while self._step():
            pass


def _build_nc():
    import concourse.bass as bass
    import concourse.mybir as mybir
    import concourse.tile as tile
    from concourse import bacc

    F32 = mybir.dt.float32
    BF16 = mybir.dt.bfloat16
    EXP = mybir.ActivationFunctionType.Exp

    nc = bacc.Bacc("TRN2", target_bir_lowering=False, debug=False,
                   num_devices=NCORES)

    xT_d = nc.dram_tensor("xT", [DQ, N], BF16, kind="ExternalInput")
    ctxT_d = nc.dram_tensor("ctxT", [DC, M], BF16, kind="ExternalInput")
    wq_d = nc.dram_tensor("wq", [DQ, CI], BF16, kind="ExternalInput")
    wk_d = nc.dram_tensor("wk", [DC, CI], BF16, kind="ExternalInput")
    wv_d = nc.dram_tensor("wv", [DC, CI], BF16, kind="ExternalInput")
    wo_d = nc.dram_tensor("wo", [CI, INNER], BF16, kind="ExternalInput")
    mb_d = nc.dram_tensor("mask01", [MT, PT], F32, kind="ExternalInput")
    out_d = nc.dram_tensor("out", [N, INNER], F32, kind="ExternalOutput")

    with tile.TileContext(nc) as tc:
      with (
          tc.tile_pool(name="persist", bufs=1) as pp,
          tc.tile_pool(name="ctx_s", bufs=2) as pcs,
          tc.tile_pool(name="xt_s", bufs=2) as pxs,
          tc.tile_pool(name="pe_p", bufs=3) as ppe,
          tc.tile_pool(name="small", bufs=2) as psm,
          tc.tile_pool(name="dout", bufs=3) as pdo,
          tc.tile_pool(name="sps_p", bufs=2, space="PSUM") as sps_p,
          tc.tile_pool(name="oacc", bufs=1, space="PSUM") as oacc_p,
          tc.tile_pool(name="aux", bufs=2, space="PSUM") as aux_p,
      ):
        kt = [pp.tile([PT, M], BF16, tag=f"kt{p}", name=f"kt{p}")
              for p in range(4)]
        vt = [pp.tile([PT, HPC * (DH + 1)], BF16, tag=f"vt{t}", name=f"vt{t}")
              for t in range(MT)]
        qt = [pp.tile([PT, N], BF16, tag=f"qt{p}", name=f"qt{p}")
              for p in range(4)]
        ot = [pp.tile([PT, N], BF16, tag=f"ot{p}", name=f"ot{p}")
              for p in range(4)]
        mask_t = pp.tile([PT, MT], F32, tag="mask", name="mask")
        wq_t = [pp.tile([PT, CI], BF16, tag=f"wq{k}", name=f"wq{k}")
                for k in range(KT_DQ)]
        wk_t = [pp.tile([PT, CI], BF16, tag=f"wk{k}", name=f"wk{k}")
                for k in range(KT_DQ)]
        wv_t = [pp.tile([PT, CI], BF16, tag=f"wv{k}", name=f"wv{k}")
                for k in range(KT_DQ)]
        wo_t = [pp.tile([PT, INNER], BF16, tag=f"wo{k}", name=f"wo{k}")
                for k in range(4)]

        wq = WorkQueue()

        # ---------------- staging DMA helpers ----------------
        ctx_tiles = {}   # q -> list of 8 tiles

        def dma_ctx(q):
            ts = []
            for k in range(KT_DQ):
                c = pcs.tile([PT, NB], BF16, tag=f"ctx{k}", name=f"ctx{k}")
                nc.sync.dma_start(c[:], ctxT_d[k * PT:(k + 1) * PT,
                                               q * NB:(q + 1) * NB])
                ts.append(c)
            ctx_tiles[q] = ts

        xt_tiles = {}    # j -> list of 8 tiles

        def dma_x(j):
            ts = []
            for k in range(KT_DQ):
                c = pxs.tile([PT, NB], BF16, tag=f"xt{k}", name=f"xt{k}")
                nc.sync.dma_start(c[:], xT_d[k * PT:(k + 1) * PT,
                                             j * NB:(j + 1) * NB])
                ts.append(c)
            xt_tiles[j] = ts

        # ---------------- background work generators ----------------
        def g_fn(fn, *a):
            def g():
                fn(*a)
                yield
            return g()

        def g_ktq(p, q):
            ctx = ctx_tiles[q]
            ps = aux_p.tile([PT, NB], F32, tag="aux", name="aux")
            for k in range(KT_DQ):
                nc.tensor.matmul(ps[:], wk_t[k][:, p * PT:(p + 1) * PT],
                                 ctx[k][:],
                                 start=(k == 0), stop=(k == KT_DQ - 1))
                if k % 2 == 1:
                    yield
            nc.vector.tensor_copy(kt[p][:, q * NB:(q + 1) * NB], ps[:])
            yield

        def g_vt(t):
            q = t // 4
            ti = t % 4
            ctx = ctx_tiles[q]
            ps = aux_p.tile([PT, CI], F32, tag="aux", name="aux")
            for k in range(KT_DQ):
                nc.tensor.matmul(ps[:], ctx[k][:, ti * PT:(ti + 1) * PT],
                                 wv_t[k][:],
                                 start=(k == 0), stop=(k == KT_DQ - 1))
                if k % 2 == 1:
                    yield
            dst = vt[t][:].rearrange("p (h c) -> p h c", c=DH + 1)
            # fold the mask into V and the ones column: masked m-rows
            # contribute 0 to both the numerator and the softmax sum
            nc.vector.tensor_scalar_mul(
                dst[:, :, 0:DH],
                ps[:].rearrange("p (h c) -> p h c", c=DH),
                mask_t[:, t:t + 1])
            nc.vector.memset(dst[:, :, DH:DH + 1], 1.0)
            nc.vector.tensor_scalar_mul(dst[:, :, DH:DH + 1],
                                        dst[:, :, DH:DH + 1],
                                        mask_t[:, t:t + 1])
            yield

        def g_qchain(p, j):
            xt = xt_tiles[j]
            ps = aux_p.tile([PT, NB], F32, tag="aux", name="aux")
            for k in range(KT_DQ):
                nc.tensor.matmul(ps[:], wq_t[k][:, p * PT:(p + 1) * PT],
                                 xt[k][:],
                                 start=(k == 0), stop=(k == KT_DQ - 1))
                if k % 2 == 1:
                    yield
            nc.vector.tensor_copy(qt[p][:, j * NB:(j + 1) * NB], ps[:])
            yield

        def g_dchunk(j, nt):
            # out rows nt*128..(nt+1)*128  =  ot[:, nt-slice].T @ Wo
            for c in range(INNER // NB):
                ps = aux_p.tile([PT, NB], F32, tag="aux", name="aux")
                for k in range(4):
                    nc.tensor.matmul(
                        ps[:], ot[k][:, nt * PT:(nt + 1) * PT],
                        wo_t[k][:, c * NB:(c + 1) * NB],
                        start=(k == 0), stop=(k == 3))
                    yield
                ob = pdo.tile([PT, NB], F32, tag="dout", name="dout")
                nc.vector.tensor_copy(ob[:], ps[:])
                nc.sync.dma_start(
                    out_d[nt * PT:(nt + 1) * PT, c * NB:(c + 1) * NB], ob[:])
            yield

        # ---------------- attention emitters ----------------
        def emit_s_exp(p, j, t):
            jq = slice(j * NB, (j + 1) * NB)
            sps = sps_p.tile([PT, 2 * NB], F32, tag="sps", name="sps")
            nc.tensor.matmul(sps[:, 0:NB],
                             kt[p][0:DH, t * PT:(t + 1) * PT],
                             qt[p][0:DH, jq], start=True, stop=True)
            nc.tensor.matmul(sps[:, NB:2 * NB],
                             kt[p][DH:2 * DH, t * PT:(t + 1) * PT],
                             qt[p][DH:2 * DH, jq], start=True, stop=True)
            pe = ppe.tile([PT, 2 * NB], BF16, tag="pe", name="pe")
            nc.scalar.activation(pe[:], sps[:], EXP, scale=SCALE)
            return pe

        def emit_av(pes, oA, oB, hA, hB, t):
            nc.tensor.matmul(oA[:],
                             vt[t][:, hA * (DH + 1):(hA + 1) * (DH + 1)],
                             pes[:, 0:NB],
                             start=(t == 0), stop=(t == MT - 1))
            nc.tensor.matmul(oB[:],
                             vt[t][:, hB * (DH + 1):(hB + 1) * (DH + 1)],
                             pes[:, NB:2 * NB],
                             start=(t == 0), stop=(t == MT - 1))

        def emit_normalize(prev):
            # stage oA/oB out to SBUF first: oacc has bufs=1, so the psum
            # must be free before the next window's first AV; everything
            # after the two staging copies is off the critical path
            p, j, oA, oB = prev
            jq = slice(j * NB, (j + 1) * NB)
            ocA = psm.tile([DH + 1, NB], F32, tag="ocA", name="ocA")
            ocB = psm.tile([DH + 1, NB], F32, tag="ocB", name="ocB")
            nc.vector.tensor_copy(ocA[:], oA[:])
            nc.vector.tensor_copy(ocB[:], oB[:])
            sums = psm.tile([1, 2 * NB], F32, tag="sums", name="sums")
            nc.vector.tensor_copy(sums[0:1, 0:NB], ocA[DH:DH + 1, :])
            nc.vector.tensor_copy(sums[0:1, NB:2 * NB], ocB[DH:DH + 1, :])
            rr = psm.tile([1, 2 * NB], F32, tag="rr", name="rr")
            nc.vector.reciprocal_approx_fast(rr[0:1, :], sums[0:1, :])
            bcs = psm.tile([DH, 2 * NB], F32, tag="bcs", name="bcs")
            nc.gpsimd.partition_broadcast(bcs[:], rr[0:1, :])
            nc.vector.tensor_mul(ot[p][0:DH, jq], ocA[0:DH, :], bcs[:, 0:NB])
            tmpB = psm.tile([DH, NB], BF16, tag="tmpB", name="tmpB")
            nc.vector.tensor_mul(tmpB[:], ocB[0:DH, :], bcs[:, NB:2 * NB])
            nc.sync.dma_start(ot[p][DH:2 * DH, jq], tmpB[:])

        # ---------------- emission ----------------
        # DMAs: exp-critical first
        for t in range(MT):
            nc.sync.dma_start(mask_t[:, t:t + 1], mb_d[t, :])
        for k in range(KT_DQ):
            nc.sync.dma_start(wk_t[k][:], wk_d[k * PT:(k + 1) * PT, :])
        dma_ctx(0)
        for k in range(KT_DQ):
            nc.sync.dma_start(wv_t[k][:], wv_d[k * PT:(k + 1) * PT, :])
            nc.sync.dma_start(wq_t[k][:], wq_d[k * PT:(k + 1) * PT, :])
        dma_x(0)
        for k in range(4):
            nc.sync.dma_start(wo_t[k][:], wo_d[k * PT:(k + 1) * PT, :])

        # prologue: just enough for (j0, p0, t=0..3)
        wq.add(("ktq", 0, 0), g_ktq(0, 0))
        for t in range(4):
            wq.add(("vt", t), g_vt(t))
        wq.add(("qt", 0, 0), g_qchain(0, 0))
        # j0 A-work + Q chains, quarter-grouped
        for p in range(1, 4):
            wq.add(("ktq", p, 0), g_ktq(p, 0))
            wq.add(("qt", p, 0), g_qchain(p, 0))
        for q in range(1, 4):
            wq.add(("dma_ctx", q), g_fn(dma_ctx, q))
            wq.add(("ktq", 0, q), g_ktq(0, q))
            for t in range(4 * q, 4 * q + 4):
                wq.add(("vt", t), g_vt(t))
            for p in range(1, 4):
                wq.add(("ktq", p, q), g_ktq(p, q))
        # Q chains for j1..3 (x DMA ahead of each group)
        for j in range(1, NJ):
            wq.add(("dma_x", j), g_fn(dma_x, j))
            for p in range(4):
                wq.add(("qt", p, j), g_qchain(p, j))

        prev = None
        for j in range(NJ):
            for p in range(4):
                hA, hB = 2 * p, 2 * p + 1
                wq.drain(("qt", p, j))
                oA = oacc_p.tile([DH + 1, NB], F32, tag="oA", name="oA")
                oB = oacc_p.tile([DH + 1, NB], F32, tag="oB", name="oB")
                pes = [None] * MT
                for t in range(MT):
                    if j == 0:
                        wq.drain(("ktq", p, t // 4))
                        if p == 0:
                            wq.drain(("vt", t))
                    pes[t] = emit_s_exp(p, j, t)
                    # oacc has bufs=1: normalize (the reader of the previous
                    # window's oA/oB) must be emitted before this window's
                    # first AV (their overwriter) lands at t==1
                    if t == 0 and prev is not None:
                        emit_normalize(prev)
                        prev = None
                    if t == 3 and p == 0 and j > 0:
                        # ot[*][:, (j-1)-block] all normalized now
                        for nt in range(4 * (j - 1), 4 * j):
                            wq.add(("D", nt), g_dchunk(j - 1, nt))
                    if t >= 1:
                        emit_av(pes[t - 1], oA, oB, hA, hB, t - 1)
                        pes[t - 1] = None
                    if j > 0 or p > 0:
                        wq.pump(1)
                emit_av(pes[MT - 1], oA, oB, hA, hB, MT - 1)
                prev = (p, j, oA, oB)
        emit_normalize(prev)
        for nt in range(4 * (NJ - 1), 4 * NJ):
            wq.add(("D", nt), g_dchunk(NJ - 1, nt))
        wq.drain_all()

    nc.compile()
    return nc


def _get_nc():
    if "nc" not in _CACHE:
        _CACHE["nc"] = _build_nc()
    return _CACHE["nc"]


def make_in_maps(x, context, mask, Wq, Wk, Wv, Wo):
    import ml_dtypes
    bf16 = ml_dtypes.bfloat16
    x = np.asarray(x, np.float32)
    context = np.asarray(context, np.float32)
    mask = np.asarray(mask)
    mask01 = np.where(mask, np.float32(1.0), np.float32(0.0))
    wqs, wks, wvs, wos = [], [], [], []
    for g in range(HG):
        cs = slice(g * CI, (g + 1) * CI)
        wqs.append(np.ascontiguousarray(
            np.asarray(Wq, np.float32)[:, cs].astype(bf16)))
        wks.append(np.ascontiguousarray(
            np.asarray(Wk, np.float32)[:, cs].astype(bf16)))
        wvs.append(np.ascontiguousarray(
            np.asarray(Wv, np.float32)[:, cs].astype(bf16)))
        wos.append(np.ascontiguousarray(
            np.asarray(Wo, np.float32)[cs, :].astype(bf16)))
    in_maps = []
    for b in range(B):
        xT = np.ascontiguousarray(x[b].T.astype(bf16))
        ctxT = np.ascontiguousarray(context[b].T.astype(bf16))
        mb = np.ascontiguousarray(mask01[b].reshape(MT, PT))
        for g in range(HG):
            in_maps.append({
                "xT": xT, "ctxT": ctxT,
                "wq": wqs[g], "wk": wks[g], "wv": wvs[g], "wo": wos[g],
                "mask01": mb,
            })
    return in_maps


def combine(results, bo):
    bo = np.asarray(bo, np.float32)
    out = np.empty((B, N, INNER), np.float32)
    for b in range(B):
        out[b] = (results[2 * b]["out"] + results[2 * b + 1]["out"]
                  + bo[None, :])
    return out


def kernel(x, context, mask, Wq, Wk, Wv, Wo, bo):
    from concourse import bass2jax
    nc = _get_nc()
    in_maps = make_in_maps(x, context, mask, Wq, Wk, Wv, Wo)
    results = bass2jax.run_bass_via_pjrt(nc, in_maps, n_cores=NCORES)
    return combine(results, bo)


# revision 7
# speedup vs baseline: 1.1806x; 1.0290x over previous
"""Fused cross-attention kernel for TRN2, sharded over 8 NeuronCores.

Sharding: core = 2*b + g  (b = batch 0..3 data-parallel, g = head-group 0..1
tensor-parallel over heads: heads g*8..g*8+7, i.e. columns g*512..(g+1)*512 of
Wq/Wk/Wv and rows g*512..(g+1)*512 of Wo). Each core computes a partial
out = softmax((x@Wq)(ctx@Wk)^T/sqrt(d)) (ctx@Wv) @ Wo_slice for its batch;
the host sums the two head-group partials per batch and adds bo.

Schedule: the ScalarE exp stream (256 x [128,1024] activations ~ 285us) is
the critical resource; everything else hides under it.  Loops run j (n-block)
outer, pair inner, m-tile innermost.  All non-attention PE work (K^T/V/Q
projections, out = O^T.T@Wo) is emitted through a work queue that drips ~1
matmul per t-step into the PE queue, plus watermark draining so the first
j-block can start ~15us in while K/V production continues underneath.
S pairs are row-tiled (heads at PE row tiles 0/64) and stream concurrently.
Mask is folded into V and the ones-column (masked rows contribute 0 to both
numerator and softmax sum), so exp needs no bias operand.  Normalize uses
DVE + a 0-stride DMA partition-broadcast (no gpsimd).
"""
import numpy as np

B, N, M = 4, 2048, 2048
DQ = 1024
DC = 1024
H = 16
DH = 64
INNER = 1024
HG = 2            # head groups (tensor parallel)
HPC = H // HG     # heads per core
CI = HPC * DH     # 512 inner dims per core
NCORES = 8
PT = 128          # partition tile
NB = 512          # n-block
KT_DQ = DQ // PT  # 8 contraction tiles for projections
MT = M // PT      # 16 m-tiles
NT = N // PT      # 16 n-tiles
NJ = N // NB      # 4 n-blocks
SCALE = DH ** -0.5

_CACHE = {}


class WorkQueue:
    """Ordered generators of background PE work, dripped into the emission
    stream.  pump(n) advances n yield-units; drain(tag) runs until the
    generator registered under tag has completed."""

    def __init__(self):
        self.items = []      # list of (tag, generator)
        self.done = set()
        self.active = None   # (tag, gen)

    def add(self, tag, gen):
        self.items.append((tag, gen))

    def _step(self):
        # advance the current generator by one unit; True if work remains
        if self.active is None:
            if not self.items:
                return False
            self.active = self.items.pop(0)
        tag, gen = self.active
        try:
            next(gen)
        except StopIteration:
            self.done.add(tag)
            self.active = None
        return True

    def pump(self, n):
        for _ in range(n):
            if not self._step():
                return

    def drain(self, tag):
        while tag not in self.done:
            if not self._step():
                raise RuntimeError(f"work item {tag} never registered")

    def drain_all(self):
        while self._step():
            pass


def _build_nc():
    import concourse.bass as bass
    import concourse.mybir as mybir
    import concourse.tile as tile
    from concourse import bacc

    F32 = mybir.dt.float32
    BF16 = mybir.dt.bfloat16
    EXP = mybir.ActivationFunctionType.Exp

    nc = bacc.Bacc("TRN2", target_bir_lowering=False, debug=False,
                   num_devices=NCORES)

    xT_d = nc.dram_tensor("xT", [DQ, N], BF16, kind="ExternalInput")
    ctxT_d = nc.dram_tensor("ctxT", [DC, M], BF16, kind="ExternalInput")
    wq_d = nc.dram_tensor("wq", [DQ, CI], BF16, kind="ExternalInput")
    wk_d = nc.dram_tensor("wk", [DC, CI], BF16, kind="ExternalInput")
    wv_d = nc.dram_tensor("wv", [DC, CI], BF16, kind="ExternalInput")
    wo_d = nc.dram_tensor("wo", [CI, INNER], BF16, kind="ExternalInput")
    mb_d = nc.dram_tensor("mask01", [PT, MT], F32, kind="ExternalInput")
    out_d = nc.dram_tensor("out", [N, INNER], F32, kind="ExternalOutput")

    with tile.TileContext(nc) as tc:
      with (
          tc.tile_pool(name="persist", bufs=1) as pp,
          tc.tile_pool(name="ctx_s", bufs=2) as pcs,
          tc.tile_pool(name="xt_s", bufs=2) as pxs,
          tc.tile_pool(name="pe_p", bufs=3) as ppe,
          tc.tile_pool(name="small", bufs=2) as psm,
          tc.tile_pool(name="dout", bufs=3) as pdo,
          tc.tile_pool(name="sps_p", bufs=2, space="PSUM") as sps_p,
          tc.tile_pool(name="oacc", bufs=1, space="PSUM") as oacc_p,
          tc.tile_pool(name="aux", bufs=2, space="PSUM") as aux_p,
      ):
        kt = [pp.tile([PT, M], BF16, tag=f"kt{p}", name=f"kt{p}")
              for p in range(4)]
        vt = [pp.tile([PT, HPC * (DH + 1)], BF16, tag=f"vt{t}", name=f"vt{t}")
              for t in range(MT)]
        qt = [pp.tile([PT, N], BF16, tag=f"qt{p}", name=f"qt{p}")
              for p in range(4)]
        ot = [pp.tile([PT, N], BF16, tag=f"ot{p}", name=f"ot{p}")
              for p in range(4)]
        mask_t = pp.tile([PT, MT], F32, tag="mask", name="mask")
        wq_all = pp.tile([PT, KT_DQ * CI], BF16, tag="wq_all", name="wq_all")
        wk_all = pp.tile([PT, KT_DQ * CI], BF16, tag="wk_all", name="wk_all")
        wv_all = pp.tile([PT, KT_DQ * CI], BF16, tag="wv_all", name="wv_all")
        wo_all = pp.tile([PT, 4 * INNER], BF16, tag="wo_all", name="wo_all")
        wq_t = [wq_all[:, k * CI:(k + 1) * CI] for k in range(KT_DQ)]
        wk_t = [wk_all[:, k * CI:(k + 1) * CI] for k in range(KT_DQ)]
        wv_t = [wv_all[:, k * CI:(k + 1) * CI] for k in range(KT_DQ)]
        wo_t = [wo_all[:, k * INNER:(k + 1) * INNER] for k in range(4)]

        wq = WorkQueue()

        # ---------------- staging DMA helpers ----------------
        ctx_tiles = {}   # q -> list of 8 tiles

        def dma_ctx(q):
            ca = pcs.tile([PT, KT_DQ * NB], BF16, tag="ctxq", name="ctxq")
            nc.sync.dma_start(
                ca[:].rearrange("p (k f) -> p k f", k=KT_DQ),
                ctxT_d[:, q * NB:(q + 1) * NB]
                .rearrange("(k p) f -> p k f", k=KT_DQ))
            ctx_tiles[q] = [ca[:, k * NB:(k + 1) * NB] for k in range(KT_DQ)]

        xt_tiles = {}    # j -> list of 8 tiles

        def dma_x(j):
            xa = pxs.tile([PT, KT_DQ * NB], BF16, tag="xq", name="xq")
            nc.sync.dma_start(
                xa[:].rearrange("p (k f) -> p k f", k=KT_DQ),
                xT_d[:, j * NB:(j + 1) * NB]
                .rearrange("(k p) f -> p k f", k=KT_DQ))
            xt_tiles[j] = [xa[:, k * NB:(k + 1) * NB] for k in range(KT_DQ)]

        # ---------------- background work generators ----------------
        def g_fn(fn, *a):
            def g():
                fn(*a)
                yield
            return g()

        def g_ktq(p, q):
            ctx = ctx_tiles[q]
            ps = aux_p.tile([PT, NB], F32, tag="aux", name="aux")
            for k in range(KT_DQ):
                nc.tensor.matmul(ps[:], wk_t[k][:, p * PT:(p + 1) * PT],
                                 ctx[k],
                                 start=(k == 0), stop=(k == KT_DQ - 1))
                if k % 2 == 1:
                    yield
            nc.vector.tensor_copy(kt[p][:, q * NB:(q + 1) * NB], ps[:])
            yield

        def g_vt(t):
            q = t // 4
            ti = t % 4
            ctx = ctx_tiles[q]
            ps = aux_p.tile([PT, CI], F32, tag="aux", name="aux")
            for k in range(KT_DQ):
                nc.tensor.matmul(ps[:], ctx[k][:, ti * PT:(ti + 1) * PT],
                                 wv_t[k][:],
                                 start=(k == 0), stop=(k == KT_DQ - 1))
                if k % 2 == 1:
                    yield
            dst = vt[t][:].rearrange("p (h c) -> p h c", c=DH + 1)
            # fold the mask into V and the ones column: masked m-rows
            # contribute 0 to both the numerator and the softmax sum
            nc.vector.tensor_scalar_mul(
                dst[:, :, 0:DH],
                ps[:].rearrange("p (h c) -> p h c", c=DH),
                mask_t[:, t:t + 1])
            nc.vector.memset(dst[:, :, DH:DH + 1], 1.0)
            nc.vector.tensor_scalar_mul(dst[:, :, DH:DH + 1],
                                        dst[:, :, DH:DH + 1],
                                        mask_t[:, t:t + 1])
            yield

        def g_qchain(p, j):
            xt = xt_tiles[j]
            ps = aux_p.tile([PT, NB], F32, tag="aux", name="aux")
            for k in range(KT_DQ):
                nc.tensor.matmul(ps[:], wq_t[k][:, p * PT:(p + 1) * PT],
                                 xt[k],
                                 start=(k == 0), stop=(k == KT_DQ - 1))
                if k % 2 == 1:
                    yield
            nc.vector.tensor_copy(qt[p][:, j * NB:(j + 1) * NB], ps[:])
            yield

        def g_dchunk(j, nt):
            # out rows nt*128..(nt+1)*128  =  ot[:, nt-slice].T @ Wo
            ob = pdo.tile([PT, INNER], F32, tag="dout", name="dout")
            for c in range(INNER // NB):
                ps = aux_p.tile([PT, NB], F32, tag="aux", name="aux")
                for k in range(4):
                    nc.tensor.matmul(
                        ps[:], ot[k][:, nt * PT:(nt + 1) * PT],
                        wo_t[k][:, c * NB:(c + 1) * NB],
                        start=(k == 0), stop=(k == 3))
                    yield
                nc.vector.tensor_copy(ob[:, c * NB:(c + 1) * NB], ps[:])
            nc.sync.dma_start(out_d[nt * PT:(nt + 1) * PT, :], ob[:])
            yield

        # ---------------- attention emitters ----------------
        def emit_s_exp(p, j, t):
            jq = slice(j * NB, (j + 1) * NB)
            sps = sps_p.tile([PT, 2 * NB], F32, tag="sps", name="sps")
            nc.tensor.matmul(sps[:, 0:NB],
                             kt[p][0:DH, t * PT:(t + 1) * PT],
                             qt[p][0:DH, jq], start=True, stop=True)
            nc.tensor.matmul(sps[:, NB:2 * NB],
                             kt[p][DH:2 * DH, t * PT:(t + 1) * PT],
                             qt[p][DH:2 * DH, jq], start=True, stop=True)
            pe = ppe.tile([PT, 2 * NB], BF16, tag="pe", name="pe")
            nc.scalar.activation(pe[:], sps[:], EXP, scale=SCALE)
            return pe

        def emit_av(pes, oA, oB, hA, hB, t):
            nc.tensor.matmul(oA[:],
                             vt[t][:, hA * (DH + 1):(hA + 1) * (DH + 1)],
                             pes[:, 0:NB],
                             start=(t == 0), stop=(t == MT - 1))
            nc.tensor.matmul(oB[:],
                             vt[t][:, hB * (DH + 1):(hB + 1) * (DH + 1)],
                             pes[:, NB:2 * NB],
                             start=(t == 0), stop=(t == MT - 1))

        def emit_normalize(prev):
            # stage oA/oB out to SBUF first: oacc has bufs=1, so the psum
            # must be free before the next window's first AV; everything
            # after the two staging copies is off the critical path
            p, j, oA, oB = prev
            jq = slice(j * NB, (j + 1) * NB)
            ocA = psm.tile([DH + 1, NB], F32, tag="ocA", name="ocA")
            ocB = psm.tile([DH + 1, NB], F32, tag="ocB", name="ocB")
            nc.vector.tensor_copy(ocA[:], oA[:])
            nc.vector.tensor_copy(ocB[:], oB[:])
            sums = psm.tile([1, 2 * NB], F32, tag="sums", name="sums")
            nc.vector.tensor_copy(sums[0:1, 0:NB], ocA[DH:DH + 1, :])
            nc.vector.tensor_copy(sums[0:1, NB:2 * NB], ocB[DH:DH + 1, :])
            rr = psm.tile([1, 2 * NB], F32, tag="rr", name="rr")
            nc.vector.reciprocal_approx_fast(rr[0:1, :], sums[0:1, :])
            bcs = psm.tile([DH, 2 * NB], F32, tag="bcs", name="bcs")
            nc.gpsimd.partition_broadcast(bcs[:], rr[0:1, :])
            nc.vector.tensor_mul(ot[p][0:DH, jq], ocA[0:DH, :], bcs[:, 0:NB])
            tmpB = psm.tile([DH, NB], BF16, tag="tmpB", name="tmpB")
            nc.vector.tensor_mul(tmpB[:], ocB[0:DH, :], bcs[:, NB:2 * NB])
            nc.sync.dma_start(ot[p][DH:2 * DH, jq], tmpB[:])

        # ---------------- emission ----------------
        # DMAs: exp-critical first, one coalesced DMA per tensor (each
        # trigger costs ~650ns of SP-queue time, so fewer is faster)
        nc.sync.dma_start(
            wk_all[:].rearrange("p (k f) -> p k f", k=KT_DQ),
            wk_d[:, :].rearrange("(k p) f -> p k f", k=KT_DQ))
        dma_ctx(0)
        nc.sync.dma_start(
            wq_all[:].rearrange("p (k f) -> p k f", k=KT_DQ),
            wq_d[:, :].rearrange("(k p) f -> p k f", k=KT_DQ))
        dma_x(0)
        nc.sync.dma_start(mask_t[:], mb_d[:, :])
        nc.sync.dma_start(
            wv_all[:].rearrange("p (k f) -> p k f", k=KT_DQ),
            wv_d[:, :].rearrange("(k p) f -> p k f", k=KT_DQ))
        nc.sync.dma_start(
            wo_all[:].rearrange("p (k f) -> p k f", k=4),
            wo_d[:, :].rearrange("(k p) f -> p k f", k=4))

        # prologue: just enough for (j0, p0, t=0..3)
        wq.add(("ktq", 0, 0), g_ktq(0, 0))
        wq.add(("qt", 0, 0), g_qchain(0, 0))
        for t in range(4):
            wq.add(("vt", t), g_vt(t))
        # j0 A-work + Q chains, quarter-grouped
        for p in range(1, 4):
            wq.add(("ktq", p, 0), g_ktq(p, 0))
            wq.add(("qt", p, 0), g_qchain(p, 0))
        for q in range(1, 4):
            wq.add(("dma_ctx", q), g_fn(dma_ctx, q))
            wq.add(("ktq", 0, q), g_ktq(0, q))
            for t in range(4 * q, 4 * q + 4):
                wq.add(("vt", t), g_vt(t))
            for p in range(1, 4):
                wq.add(("ktq", p, q), g_ktq(p, q))
        # Q chains for j1..3 (x DMA ahead of each group)
        for j in range(1, NJ):
            wq.add(("dma_x", j), g_fn(dma_x, j))
            for p in range(4):
                wq.add(("qt", p, j), g_qchain(p, j))

        prev = None
        for j in range(NJ):
            for p in range(4):
                hA, hB = 2 * p, 2 * p + 1
                wq.drain(("qt", p, j))
                oA = oacc_p.tile([DH + 1, NB], F32, tag="oA", name="oA")
                oB = oacc_p.tile([DH + 1, NB], F32, tag="oB", name="oB")
                pes = [None] * MT
                for t in range(MT):
                    if j == 0:
                        wq.drain(("ktq", p, t // 4))
                        if p == 0:
                            wq.drain(("vt", t))
                    pes[t] = emit_s_exp(p, j, t)
                    # oacc has bufs=1: normalize (the reader of the previous
                    # window's oA/oB) must be emitted before this window's
                    # first AV (their overwriter) lands at t==1
                    if t == 0 and prev is not None:
                        emit_normalize(prev)
                        prev = None
                    if t == 3 and p == 0 and j > 0:
                        # ot[*][:, (j-1)-block] all normalized now
                        for nt in range(4 * (j - 1), 4 * j):
                            wq.add(("D", nt), g_dchunk(j - 1, nt))
                    if t >= 1:
                        emit_av(pes[t - 1], oA, oB, hA, hB, t - 1)
                        pes[t - 1] = None
                    if j > 0 or p > 0:
                        wq.pump(1)
                emit_av(pes[MT - 1], oA, oB, hA, hB, MT - 1)
                prev = (p, j, oA, oB)
        emit_normalize(prev)
        for nt in range(4 * (NJ - 1), 4 * NJ):
            wq.add(("D", nt), g_dchunk(NJ - 1, nt))
        wq.drain_all()

    nc.compile()
    return nc


def _get_nc():
    if "nc" not in _CACHE:
        _CACHE["nc"] = _build_nc()
    return _CACHE["nc"]


def make_in_maps(x, context, mask, Wq, Wk, Wv, Wo):
    import ml_dtypes
    bf16 = ml_dtypes.bfloat16
    x = np.asarray(x, np.float32)
    context = np.asarray(context, np.float32)
    mask = np.asarray(mask)
    mask01 = np.where(mask, np.float32(1.0), np.float32(0.0))
    wqs, wks, wvs, wos = [], [], [], []
    for g in range(HG):
        cs = slice(g * CI, (g + 1) * CI)
        wqs.append(np.ascontiguousarray(
            np.asarray(Wq, np.float32)[:, cs].astype(bf16)))
        wks.append(np.ascontiguousarray(
            np.asarray(Wk, np.float32)[:, cs].astype(bf16)))
        wvs.append(np.ascontiguousarray(
            np.asarray(Wv, np.float32)[:, cs].astype(bf16)))
        wos.append(np.ascontiguousarray(
            np.asarray(Wo, np.float32)[cs, :].astype(bf16)))
    in_maps = []
    for b in range(B):
        xT = np.ascontiguousarray(x[b].T.astype(bf16))
        ctxT = np.ascontiguousarray(context[b].T.astype(bf16))
        mb = np.ascontiguousarray(mask01[b].reshape(MT, PT).T)
        for g in range(HG):
            in_maps.append({
                "xT": xT, "ctxT": ctxT,
                "wq": wqs[g], "wk": wks[g], "wv": wvs[g], "wo": wos[g],
                "mask01": mb,
            })
    return in_maps


def combine(results, bo):
    bo = np.asarray(bo, np.float32)
    out = np.empty((B, N, INNER), np.float32)
    for b in range(B):
        out[b] = (results[2 * b]["out"] + results[2 * b + 1]["out"]
                  + bo[None, :])
    return out


def kernel(x, context, mask, Wq, Wk, Wv, Wo, bo):
    from concourse import bass2jax
    nc = _get_nc()
    in_maps = make_in_maps(x, context, mask, Wq, Wk, Wv, Wo)
    results = bass2jax.run_bass_via_pjrt(nc, in_maps, n_cores=NCORES)
    return combine(results, bo)


# revision 8
# speedup vs baseline: 1.2114x; 1.0261x over previous
"""Fused cross-attention kernel for TRN2, sharded over 8 NeuronCores.

Sharding: core = 2*b + g  (b = batch 0..3 data-parallel, g = head-group 0..1
tensor-parallel over heads: heads g*8..g*8+7, i.e. columns g*512..(g+1)*512 of
Wq/Wk/Wv and rows g*512..(g+1)*512 of Wo). Each core computes a partial
out = softmax((x@Wq)(ctx@Wk)^T/sqrt(d)) (ctx@Wv) @ Wo_slice for its batch;
the host sums the two head-group partials per batch and adds bo.

Schedule: the ScalarE exp stream (256 x [128,1024] activations ~ 285us) is
the critical resource; everything else hides under it.  Loops run j (n-block)
outer, pair inner, m-tile innermost.  All non-attention PE work (K^T/V/Q
projections, out = O^T.T@Wo) is emitted through a work queue that drips ~1
matmul per t-step into the PE queue, plus watermark draining so the first
j-block can start ~15us in while K/V production continues underneath.
S pairs are row-tiled (heads at PE row tiles 0/64) and stream concurrently.
Mask is folded into V and the ones-column (masked rows contribute 0 to both
numerator and softmax sum), so exp needs no bias operand.  Normalize uses
DVE + a 0-stride DMA partition-broadcast (no gpsimd).
"""
import numpy as np

B, N, M = 4, 2048, 2048
DQ = 1024
DC = 1024
H = 16
DH = 64
INNER = 1024
HG = 2            # head groups (tensor parallel)
HPC = H // HG     # heads per core
CI = HPC * DH     # 512 inner dims per core
NCORES = 8
PT = 128          # partition tile
NB = 512          # n-block
KT_DQ = DQ // PT  # 8 contraction tiles for projections
MT = M // PT      # 16 m-tiles
NT = N // PT      # 16 n-tiles
NJ = N // NB      # 4 n-blocks
SCALE = DH ** -0.5

_CACHE = {}


class WorkQueue:
    """Ordered generators of background PE work, dripped into the emission
    stream.  pump(n) advances n yield-units; drain(tag) runs until the
    generator registered under tag has completed."""

    def __init__(self):
        self.items = []      # list of (tag, generator)
        self.done = set()
        self.active = None   # (tag, gen)

    def add(self, tag, gen):
        self.items.append((tag, gen))

    def _step(self):
        # advance the current generator by one unit; True if work remains
        if self.active is None:
            if not self.items:
                return False
            self.active = self.items.pop(0)
        tag, gen = self.active
        try:
            next(gen)
        except StopIteration:
            self.done.add(tag)
            self.active = None
        return True

    def pump(self, n):
        for _ in range(n):
            if not self._step():
                return

    def drain(self, tag):
        while tag not in self.done:
            if not self._step():
                raise RuntimeError(f"work item {tag} never registered")

    def drain_all(self):
        while self._step():
            pass


def _build_nc():
    import concourse.bass as bass
    import concourse.mybir as mybir
    import concourse.tile as tile
    from concourse import bacc

    F32 = mybir.dt.float32
    BF16 = mybir.dt.bfloat16
    EXP = mybir.ActivationFunctionType.Exp

    nc = bacc.Bacc("TRN2", target_bir_lowering=False, debug=False,
                   num_devices=NCORES)

    # host-prearranged: [128, ...] so every load is a 2D contiguous DMA
    xT_d = nc.dram_tensor("xT", [PT, NJ * KT_DQ * NB], BF16,
                          kind="ExternalInput")
    ctxT_d = nc.dram_tensor("ctxT", [PT, 4 * KT_DQ * NB], BF16,
                            kind="ExternalInput")
    wq_d = nc.dram_tensor("wq", [PT, KT_DQ * CI], BF16, kind="ExternalInput")
    wk_d = nc.dram_tensor("wk", [PT, KT_DQ * CI], BF16, kind="ExternalInput")
    wv_d = nc.dram_tensor("wv", [PT, KT_DQ * CI], BF16, kind="ExternalInput")
    wo_d = nc.dram_tensor("wo", [PT, 4 * INNER], BF16, kind="ExternalInput")
    mb_d = nc.dram_tensor("mask01", [PT, MT], F32, kind="ExternalInput")
    out_d = nc.dram_tensor("out", [N, INNER], F32, kind="ExternalOutput")

    with tile.TileContext(nc) as tc:
      with (
          tc.tile_pool(name="persist", bufs=1) as pp,
          tc.tile_pool(name="ctx_s", bufs=4) as pcs,
          tc.tile_pool(name="xt_s", bufs=2) as pxs,
          tc.tile_pool(name="pe_p", bufs=3) as ppe,
          tc.tile_pool(name="small", bufs=2) as psm,
          tc.tile_pool(name="dout", bufs=3) as pdo,
          tc.tile_pool(name="sps_p", bufs=2, space="PSUM") as sps_p,
          tc.tile_pool(name="oacc", bufs=1, space="PSUM") as oacc_p,
          tc.tile_pool(name="aux", bufs=2, space="PSUM") as aux_p,
      ):
        kt = [pp.tile([PT, M], BF16, tag=f"kt{p}", name=f"kt{p}")
              for p in range(4)]
        vt = [pp.tile([PT, HPC * (DH + 1)], BF16, tag=f"vt{t}", name=f"vt{t}")
              for t in range(MT)]
        qt = [pp.tile([PT, N], BF16, tag=f"qt{p}", name=f"qt{p}")
              for p in range(4)]
        ot = [pp.tile([PT, N], BF16, tag=f"ot{p}", name=f"ot{p}")
              for p in range(4)]
        mask_t = pp.tile([PT, MT], F32, tag="mask", name="mask")
        ones64 = pp.tile([1, DH], BF16, tag="ones64", name="ones64")
        wq_all = pp.tile([PT, KT_DQ * CI], BF16, tag="wq_all", name="wq_all")
        wk_all = pp.tile([PT, KT_DQ * CI], BF16, tag="wk_all", name="wk_all")
        wv_all = pp.tile([PT, KT_DQ * CI], BF16, tag="wv_all", name="wv_all")
        wo_all = pp.tile([PT, 4 * INNER], BF16, tag="wo_all", name="wo_all")
        wq_t = [wq_all[:, k * CI:(k + 1) * CI] for k in range(KT_DQ)]
        wk_t = [wk_all[:, k * CI:(k + 1) * CI] for k in range(KT_DQ)]
        wv_t = [wv_all[:, k * CI:(k + 1) * CI] for k in range(KT_DQ)]
        wo_t = [wo_all[:, k * INNER:(k + 1) * INNER] for k in range(4)]

        wq = WorkQueue()

        # ---------------- staging DMA helpers ----------------
        ctx_tiles = {}   # q -> list of 8 tiles

        def dma_ctx(q):
            ca = pcs.tile([PT, KT_DQ * NB], BF16, tag="ctxq", name="ctxq")
            nc.sync.dma_start(
                ca[:], ctxT_d[:, q * KT_DQ * NB:(q + 1) * KT_DQ * NB])
            ctx_tiles[q] = [ca[:, k * NB:(k + 1) * NB] for k in range(KT_DQ)]

        xt_tiles = {}    # j -> list of 8 tiles

        def dma_x(j):
            xa = pxs.tile([PT, KT_DQ * NB], BF16, tag="xq", name="xq")
            nc.sync.dma_start(
                xa[:], xT_d[:, j * KT_DQ * NB:(j + 1) * KT_DQ * NB])
            xt_tiles[j] = [xa[:, k * NB:(k + 1) * NB] for k in range(KT_DQ)]

        # ---------------- background work generators ----------------
        def g_fn(fn, *a):
            def g():
                fn(*a)
                yield
            return g()

        def g_ktq(p, q):
            ctx = ctx_tiles[q]
            ps = aux_p.tile([PT, NB], F32, tag="aux", name="aux")
            for k in range(KT_DQ):
                nc.tensor.matmul(ps[:], wk_t[k][:, p * PT:(p + 1) * PT],
                                 ctx[k],
                                 start=(k == 0), stop=(k == KT_DQ - 1))
                if k % 2 == 1:
                    yield
            nc.vector.tensor_copy(kt[p][:, q * NB:(q + 1) * NB], ps[:])
            yield

        def g_vt(t):
            q = t // 4
            ti = t % 4
            ctx = ctx_tiles[q]
            ps = aux_p.tile([PT, CI], F32, tag="aux", name="aux")
            for k in range(KT_DQ):
                nc.tensor.matmul(ps[:], ctx[k][:, ti * PT:(ti + 1) * PT],
                                 wv_t[k][:],
                                 start=(k == 0), stop=(k == KT_DQ - 1))
                if k % 2 == 1:
                    yield
            dst = vt[t][:].rearrange("p (h c) -> p h c", c=DH + 1)
            # fold the mask into V and the ones column: masked m-rows
            # contribute 0 to both the numerator and the softmax sum
            nc.vector.tensor_scalar_mul(
                dst[:, :, 0:DH],
                ps[:].rearrange("p (h c) -> p h c", c=DH),
                mask_t[:, t:t + 1])
            nc.vector.memset(dst[:, :, DH:DH + 1], 1.0)
            nc.vector.tensor_scalar_mul(dst[:, :, DH:DH + 1],
                                        dst[:, :, DH:DH + 1],
                                        mask_t[:, t:t + 1])
            yield

        def g_qchain(p, j):
            xt = xt_tiles[j]
            ps = aux_p.tile([PT, NB], F32, tag="aux", name="aux")
            for k in range(KT_DQ):
                nc.tensor.matmul(ps[:], wq_t[k][:, p * PT:(p + 1) * PT],
                                 xt[k],
                                 start=(k == 0), stop=(k == KT_DQ - 1))
                if k % 2 == 1:
                    yield
            nc.vector.tensor_copy(qt[p][:, j * NB:(j + 1) * NB], ps[:])
            yield

        def g_dchunk(j, nt):
            # out rows nt*128..(nt+1)*128  =  ot[:, nt-slice].T @ Wo
            ob = pdo.tile([PT, INNER], F32, tag="dout", name="dout")
            for c in range(INNER // NB):
                ps = aux_p.tile([PT, NB], F32, tag="aux", name="aux")
                for k in range(4):
                    nc.tensor.matmul(
                        ps[:], ot[k][:, nt * PT:(nt + 1) * PT],
                        wo_t[k][:, c * NB:(c + 1) * NB],
                        start=(k == 0), stop=(k == 3))
                    yield
                nc.vector.tensor_copy(ob[:, c * NB:(c + 1) * NB], ps[:])
            nc.sync.dma_start(out_d[nt * PT:(nt + 1) * PT, :], ob[:])
            yield

        # ---------------- attention emitters ----------------
        def emit_s_exp(p, j, t):
            jq = slice(j * NB, (j + 1) * NB)
            sps = sps_p.tile([PT, 2 * NB], F32, tag="sps", name="sps")
            nc.tensor.matmul(sps[:, 0:NB],
                             kt[p][0:DH, t * PT:(t + 1) * PT],
                             qt[p][0:DH, jq], start=True, stop=True)
            nc.tensor.matmul(sps[:, NB:2 * NB],
                             kt[p][DH:2 * DH, t * PT:(t + 1) * PT],
                             qt[p][DH:2 * DH, jq], start=True, stop=True)
            pe = ppe.tile([PT, 2 * NB], BF16, tag="pe", name="pe")
            nc.scalar.activation(pe[:], sps[:], EXP, scale=SCALE)
            return pe

        def emit_av(pes, oA, oB, hA, hB, t):
            nc.tensor.matmul(oA[:],
                             vt[t][:, hA * (DH + 1):(hA + 1) * (DH + 1)],
                             pes[:, 0:NB],
                             start=(t == 0), stop=(t == MT - 1))
            nc.tensor.matmul(oB[:],
                             vt[t][:, hB * (DH + 1):(hB + 1) * (DH + 1)],
                             pes[:, NB:2 * NB],
                             start=(t == 0), stop=(t == MT - 1))

        def emit_normalize(prev, last=False):
            # stage oA/oB out to SBUF first: oacc has bufs=1, so the psum
            # must be free before the next window's first AV; everything
            # after the two staging copies is off the critical path
            p, j, oA, oB = prev
            jq = slice(j * NB, (j + 1) * NB)
            ocA = psm.tile([DH + 1, NB], F32, tag="ocA", name="ocA")
            ocB = psm.tile([DH + 1, NB], F32, tag="ocB", name="ocB")
            nc.vector.tensor_copy(ocA[:], oA[:])
            nc.vector.tensor_copy(ocB[:], oB[:])
            sums = psm.tile([1, 2 * NB], F32, tag="sums", name="sums")
            nc.vector.tensor_copy(sums[0:1, 0:NB], ocA[DH:DH + 1, :])
            nc.vector.tensor_copy(sums[0:1, NB:2 * NB], ocB[DH:DH + 1, :])
            rr = psm.tile([1, 2 * NB], F32, tag="rr", name="rr")
            nc.vector.reciprocal_approx_fast(rr[0:1, :], sums[0:1, :])
            if last:
                # PE-matmul broadcast (ones64^T @ rr) keeps the PE warm into
                # phase D and skips the slow gpsimd chain at the tail
                rrb = psm.tile([1, 2 * NB], BF16, tag="rrb", name="rrb")
                nc.vector.tensor_copy(rrb[0:1, :], rr[0:1, :])
                bpA = aux_p.tile([PT, NB], F32, tag="aux", name="aux")
                bpB = aux_p.tile([PT, NB], F32, tag="aux", name="aux")
                nc.tensor.matmul(bpA[0:DH, :], ones64[0:1, :],
                                 rrb[0:1, 0:NB], start=True, stop=True)
                nc.tensor.matmul(bpB[0:DH, :], ones64[0:1, :],
                                 rrb[0:1, NB:2 * NB], start=True, stop=True)
                bA, bB = bpA[0:DH, 0:NB], bpB[0:DH, 0:NB]
            else:
                bcs = psm.tile([DH, 2 * NB], F32, tag="bcs", name="bcs")
                nc.gpsimd.partition_broadcast(bcs[:], rr[0:1, :])
                bA, bB = bcs[:, 0:NB], bcs[:, NB:2 * NB]
            nc.vector.tensor_mul(ot[p][0:DH, jq], ocA[0:DH, :], bA)
            tmpB = psm.tile([DH, NB], BF16, tag="tmpB", name="tmpB")
            nc.vector.tensor_mul(tmpB[:], ocB[0:DH, :], bB)
            nc.sync.dma_start(ot[p][DH:2 * DH, jq], tmpB[:])

        # ---------------- emission ----------------
        # DMAs: exp-critical first, one coalesced DMA per tensor (each
        # trigger costs ~650ns of SP-queue time, so fewer is faster)
        nc.vector.memset(ones64[0:1, :], 1.0)
        nc.sync.dma_start(wk_all[:], wk_d[:, :])
        dma_ctx(0)
        nc.sync.dma_start(wq_all[:], wq_d[:, :])
        dma_x(0)
        nc.sync.dma_start(mask_t[:], mb_d[:, :])
        nc.sync.dma_start(wv_all[:], wv_d[:, :])
        nc.sync.dma_start(wo_all[:], wo_d[:, :])

        # prologue: just enough for (j0, p0, t=0..3)
        wq.add(("ktq", 0, 0), g_ktq(0, 0))
        wq.add(("qt", 0, 0), g_qchain(0, 0))
        for t in range(4):
            wq.add(("vt", t), g_vt(t))
        # p0's whole diet first (its watermarks pace j0p0), other pairs after
        for q in range(1, 4):
            wq.add(("dma_ctx", q), g_fn(dma_ctx, q))
            wq.add(("ktq", 0, q), g_ktq(0, q))
            for t in range(4 * q, 4 * q + 4):
                wq.add(("vt", t), g_vt(t))
        for p in range(1, 4):
            wq.add(("ktq", p, 0), g_ktq(p, 0))
            wq.add(("qt", p, 0), g_qchain(p, 0))
            for q in range(1, 4):
                wq.add(("ktq", p, q), g_ktq(p, q))
        # Q chains for j1..3 (x DMA ahead of each group)
        for j in range(1, NJ):
            wq.add(("dma_x", j), g_fn(dma_x, j))
            for p in range(4):
                wq.add(("qt", p, j), g_qchain(p, j))

        prev = None
        for j in range(NJ):
            for p in range(4):
                hA, hB = 2 * p, 2 * p + 1
                wq.drain(("qt", p, j))
                oA = oacc_p.tile([DH + 1, NB], F32, tag="oA", name="oA")
                oB = oacc_p.tile([DH + 1, NB], F32, tag="oB", name="oB")
                pes = [None] * MT
                for t in range(MT):
                    if j == 0:
                        wq.drain(("ktq", p, t // 4))
                        if p == 0:
                            wq.drain(("vt", t))
                    pes[t] = emit_s_exp(p, j, t)
                    # oacc has bufs=1: normalize (the reader of the previous
                    # window's oA/oB) must be emitted before this window's
                    # first AV (their overwriter) lands at t==1
                    if t == 0 and prev is not None:
                        emit_normalize(prev)
                        prev = None
                    if t == 3 and p == 0 and j > 0:
                        # ot[*][:, (j-1)-block] all normalized now
                        for nt in range(4 * (j - 1), 4 * j):
                            wq.add(("D", nt), g_dchunk(j - 1, nt))
                    if t >= 1:
                        emit_av(pes[t - 1], oA, oB, hA, hB, t - 1)
                        pes[t - 1] = None
                    wq.pump(1)
                emit_av(pes[MT - 1], oA, oB, hA, hB, MT - 1)
                prev = (p, j, oA, oB)
        emit_normalize(prev, last=True)
        for nt in range(4 * (NJ - 1), 4 * NJ):
            wq.add(("D", nt), g_dchunk(NJ - 1, nt))
        wq.drain_all()

    nc.compile()
    return nc


def _get_nc():
    if "nc" not in _CACHE:
        _CACHE["nc"] = _build_nc()
    return _CACHE["nc"]


def make_in_maps(x, context, mask, Wq, Wk, Wv, Wo):
    import ml_dtypes
    bf16 = ml_dtypes.bfloat16
    x = np.asarray(x, np.float32)
    context = np.asarray(context, np.float32)
    mask = np.asarray(mask)
    mask01 = np.where(mask, np.float32(1.0), np.float32(0.0))
    def chunk_rows(a, kt):
        # [kt*128, F] -> [128, kt*F]: row k*128+p lands at [p, k*F:...]
        r, f = a.shape
        return np.ascontiguousarray(
            a.reshape(kt, PT, f).transpose(1, 0, 2).reshape(PT, kt * f))

    def quarters(aT, nq):
        # [1024, nq*512] -> [128, nq*8*512] quarter-major
        return np.ascontiguousarray(
            aT.reshape(KT_DQ, PT, nq, NB).transpose(1, 2, 0, 3)
            .reshape(PT, nq * KT_DQ * NB))

    wqs, wks, wvs, wos = [], [], [], []
    for g in range(HG):
        cs = slice(g * CI, (g + 1) * CI)
        wqs.append(chunk_rows(np.asarray(Wq, np.float32)[:, cs].astype(bf16),
                              KT_DQ))
        wks.append(chunk_rows(np.asarray(Wk, np.float32)[:, cs].astype(bf16),
                              KT_DQ))
        wvs.append(chunk_rows(np.asarray(Wv, np.float32)[:, cs].astype(bf16),
                              KT_DQ))
        wos.append(chunk_rows(np.asarray(Wo, np.float32)[cs, :].astype(bf16),
                              4))
    in_maps = []
    for b in range(B):
        xT = quarters(x[b].T.astype(bf16), NJ)
        ctxT = quarters(context[b].T.astype(bf16), 4)
        mb = np.ascontiguousarray(mask01[b].reshape(MT, PT).T)
        for g in range(HG):
            in_maps.append({
                "xT": xT, "ctxT": ctxT,
                "wq": wqs[g], "wk": wks[g], "wv": wvs[g], "wo": wos[g],
                "mask01": mb,
            })
    return in_maps


def combine(results, bo):
    bo = np.asarray(bo, np.float32)
    out = np.empty((B, N, INNER), np.float32)
    for b in range(B):
        out[b] = (results[2 * b]["out"] + results[2 * b + 1]["out"]
                  + bo[None, :])
    return out


def kernel(x, context, mask, Wq, Wk, Wv, Wo, bo):
    from concourse import bass2jax
    nc = _get_nc()
    in_maps = make_in_maps(x, context, mask, Wq, Wk, Wv, Wo)
    results = bass2jax.run_bass_via_pjrt(nc, in_maps, n_cores=NCORES)
    return combine(results, bo)
